# revision 1
# baseline (speedup 1.0000x reference)
"""Trainium2 Bass kernel for nn_Decoder (Tacotron-style attention decoder).

Sharding: pure data-parallel over batch (B=64 -> 8 cores x 8 samples),
zero collectives.  One hardware For_i loop over T=800 steps, 2 steps per
body (shares pk / enc / Wq streaming between the two steps).

Per-core per-step:
  - attention LSTMCell (x-side gates for all t precomputed on host)
  - additive attention: energy = tanh(pq + pk) over [8b,512s,1024d]
    laid out [(b,s)->partitions, d->free]; scores via fused DVE
    multiply+accumulate against replicated We; softmax without max
    subtraction (scores are O(1)); context via per-sample PE matvecs
  - decoder LSTMCell; mel projection deferred to one GEMM at the end.

Numerics: fp16 operands, fp32 PSUM/state.  h and c are stored scaled by
2 (H=2h, C=2c) so sigmoid(x)=(1+tanh(x/2))/2 becomes pure tanh and the
0.5 factors fold into host-prescaled weights.
"""

import sys

sys.path.insert(0, "/opt/trn_rl_repo")

import numpy as np

B_FULL, S, T_FULL = 64, 512, 800
ENC, DEC, MEL = 512, 1024, 80
G = 4 * DEC
NCORES = 8
BL = B_FULL // NCORES  # 8

F16 = np.float16
F32 = np.float32

RES_A = 4  # Wih_d ah-part k-tiles kept SBUF-resident (of 8)


def _perm_gates(w):
    """Reorder gate blocks i,f,g,o -> i,f,o,g along axis 0."""
    i, f, g, o = np.split(w, 4, axis=0)
    return np.concatenate([i, f, o, g], axis=0)


def _ktiles(wT):
    K, N = wT.shape
    assert K % 128 == 0
    return np.ascontiguousarray(wT.reshape(K // 128, 128, N))


def prep_host(inputs, T):
    eo = np.asarray(inputs["encoder_outputs"], F32)
    tm = np.asarray(inputs["target_mels"], F32)[:, :T]
    Wih_a = _perm_gates(np.asarray(inputs["Wih_a"], F32))
    Whh_a = _perm_gates(np.asarray(inputs["Whh_a"], F32))
    bih_a = _perm_gates(np.asarray(inputs["bih_a"], F32))
    bhh_a = _perm_gates(np.asarray(inputs["bhh_a"], F32))
    Wq = np.asarray(inputs["Wq"], F32)
    Wk = np.asarray(inputs["Wk"], F32)
    We = np.asarray(inputs["We"], F32)
    Wih_d = _perm_gates(np.asarray(inputs["Wih_d"], F32))
    Whh_d = _perm_gates(np.asarray(inputs["Whh_d"], F32))
    bih_d = _perm_gates(np.asarray(inputs["bih_d"], F32))
    bhh_d = _perm_gates(np.asarray(inputs["bhh_d"], F32))
    Wmel = np.asarray(inputs["Wmel"], F32)
    bmel = np.asarray(inputs["bmel"], F32)

    bias_d = bih_d + bhh_d
    has_bias_d = bool(np.any(bias_d))

    prev = np.concatenate([np.zeros((B_FULL, 1, MEL), F32), tm[:, :-1]], axis=1)
    enc_last = eo[:, -1, :]
    xga = (
        np.einsum("btm,gm->btg", prev, Wih_a[:, :MEL])
        + (enc_last @ Wih_a[:, MEL:].T)[:, None, :]
        + (bih_a + bhh_a)[None, None, :]
    )  # [B, T, G]

    pk = np.einsum("bse,de->bsd", eo, Wk).astype(F32)  # [B, S, DEC]

    whhaT = _ktiles((0.5 * Whh_a).T.astype(F16))
    wqT = _ktiles((0.5 * Wq).T.astype(F16))
    wihdaT = _ktiles((0.5 * Wih_d[:, :DEC]).T.astype(F16))
    wihdcT = _ktiles(Wih_d[:, DEC:].T.astype(F16))
    whhdT = _ktiles((0.5 * Whh_d).T.astype(F16))
    wmelT = (
        _ktiles((0.5 * Wmel).T.astype(F16)).transpose(1, 0, 2).reshape(128, 8 * MEL)
    )
    werep = np.ascontiguousarray(np.broadcast_to(We[0].astype(F16), (128, DEC)))
    ident = np.eye(128, dtype=F16)
    ones_r = np.ones((33, 128), F16)
    ones_c = np.ones((128, 1), F16)

    meta = dict(T=T, has_bias_d=has_bias_d)
    per_core = []
    for c in range(NCORES):
        bs = slice(c * BL, (c + 1) * BL)
        pk2 = np.ascontiguousarray(
            pk[bs].reshape(BL, 4, 128, DEC).transpose(0, 2, 1, 3).astype(F16))
        enc_c = np.ascontiguousarray(
            eo[bs].reshape(BL, 4, 128, ENC).transpose(0, 2, 1, 3).astype(F16))
        d = {
            "xga": np.ascontiguousarray(
                xga[bs].transpose(1, 0, 2).reshape(T * BL, G).astype(F16)),
            "pk2": pk2,
            "encb": enc_c,
            "whha": whhaT,
            "wq": wqT,
            "wihda": wihdaT,
            "wihdc": wihdcT,
            "whhd": whhdT,
            "werep": werep,
            "wmelT": wmelT,
            "bmel": bmel.astype(F32).reshape(MEL, 1),
            "ident": ident,
            "ones_r": ones_r,
            "ones_c": ones_c,
        }
        if has_bias_d:
            d["biasd"] = bias_d.astype(F16)[None, :]
        per_core.append(d)
    return meta, per_core


def build_program(T, has_bias_d):
    import concourse.bass as bass
    import concourse.mybir as mybir
    from concourse import bacc
    from concourse.tile import TileContext
    from concourse.bass import ds

    f16 = mybir.dt.float16
    f32 = mybir.dt.float32
    AF = mybir.ActivationFunctionType
    ALU = mybir.AluOpType
    AX = mybir.AxisListType

    assert T % 2 == 0

    nc = bacc.Bacc("TRN2", target_bir_lowering=False, debug=False,
                   num_devices=NCORES)

    dr_xga = nc.dram_tensor("xga", [T * BL, G], f16, kind="ExternalInput")
    dr_pk2 = nc.dram_tensor("pk2", [BL, 128, 4, DEC], f16, kind="ExternalInput")
    dr_enc = nc.dram_tensor("encb", [BL, 128, 4, ENC], f16, kind="ExternalInput")
    dr_whha = nc.dram_tensor("whha", [8, 128, G], f16, kind="ExternalInput")
    dr_wq = nc.dram_tensor("wq", [8, 128, DEC], f16, kind="ExternalInput")
    dr_wihda = nc.dram_tensor("wihda", [8, 128, G], f16, kind="ExternalInput")
    dr_wihdc = nc.dram_tensor("wihdc", [4, 128, G], f16, kind="ExternalInput")
    dr_whhd = nc.dram_tensor("whhd", [8, 128, G], f16, kind="ExternalInput")
    dr_werep = nc.dram_tensor("werep", [128, DEC], f16, kind="ExternalInput")
    dr_wmelT = nc.dram_tensor("wmelT", [128, 8 * MEL], f16, kind="ExternalInput")
    dr_bmel = nc.dram_tensor("bmel", [MEL, 1], f32, kind="ExternalInput")
    dr_ident = nc.dram_tensor("ident", [128, 128], f16, kind="ExternalInput")
    dr_ones_r = nc.dram_tensor("ones_r", [33, 128], f16, kind="ExternalInput")
    dr_ones_c = nc.dram_tensor("ones_c", [128, 1], f16, kind="ExternalInput")
    if has_bias_d:
        dr_biasd = nc.dram_tensor("biasd", [1, G], f16, kind="ExternalInput")
    dr_dhT = nc.dram_tensor("dhT", [T * 128, 64], f16)  # internal
    dr_mel = nc.dram_tensor("melT", [MEL, T * BL], f32, kind="ExternalOutput")

    with TileContext(nc) as tc, \
            tc.tile_pool(name="singles", bufs=1) as singles:
        # ---- residents (const-pack: ident | ones_c | werep | wmelT) ----
        wihda_sb = singles.tile([128, RES_A * G], f16, tag="wihda_sb", name="wihda_sb")
        whhd_sb = singles.tile([128, 8 * G], f16, tag="whhd_sb", name="whhd_sb")
        cpack = singles.tile([128, 1793], f16, tag="cpack", name="cpack")
        ident_sb = cpack[:, 0:128]
        ones_c_sb = cpack[:, 128:129]
        werep_sb = cpack[:, 129:129 + DEC]
        wmelT_sb = cpack[:, 1153:1153 + 8 * MEL]
        bmel_sb = singles.tile([MEL, 1], f32, tag="bmel_sb", name="bmel_sb")
        ones_r_sb = singles.tile([33, 128], f16, tag="ones_r_sb", name="ones_r_sb")
        pq_row = singles.tile([33, 8 * DEC], f16, tag="pq_row", name="pq_row")
        if has_bias_d:
            biasd_sb = singles.tile([1, G], f16, tag="biasd_sb", name="biasd_sb")
            nc.sync.dma_start(biasd_sb[:], dr_biasd[:])
        # recurrent state (slot u written at sub-step u)
        ahT = singles.tile([128, 2, 64], f16, tag="ahT", name="ahT")
        dhT = singles.tile([128, 2, 64], f16, tag="dhT", name="dhT")
        Ca = singles.tile([BL, DEC], f32, tag="Ca", name="Ca")
        Cd = singles.tile([BL, DEC], f32, tag="Cd", name="Cd")

        nc.sync.dma_start(ident_sb, dr_ident[:])
        nc.sync.dma_start(ones_c_sb, dr_ones_c[:])
        nc.sync.dma_start(werep_sb, dr_werep[:])
        nc.sync.dma_start(wmelT_sb, dr_wmelT[:])
        nc.sync.dma_start(bmel_sb[:], dr_bmel[:])
        nc.sync.dma_start(ones_r_sb[:], dr_ones_r[:])
        for j in range(RES_A):
            nc.sync.dma_start(wihda_sb[:, j * G:(j + 1) * G], dr_wihda[j])
        for j in range(8):
            nc.sync.dma_start(whhd_sb[:, j * G:(j + 1) * G], dr_whhd[j])
        nc.vector.memset(ahT[:], 0.0)
        nc.vector.memset(dhT[:], 0.0)
        nc.vector.memset(Ca[:], 0.0)
        nc.vector.memset(Cd[:], 0.0)

        # ---- per-step scratch ----
        t_sb = singles.tile([BL, G], f16, tag="t_sb", name="t_sb")
        thn = singles.tile([BL, 2, DEC], f16, tag="thn", name="thn")
        tanhc, Hnew = thn[:, 0, :], thn[:, 1, :]
        As_ = singles.tile([BL, DEC], f32, tag="As_", name="As_")
        Bs_ = singles.tile([BL, DEC], f32, tag="Bs_", name="Bs_")
        pq_sb = singles.tile([BL, 2, DEC], f16, tag="pq_sb", name="pq_sb")
        sc_sb = singles.tile([128, 2, 32], f32, tag="sc_sb", name="sc_sb")
        e3 = singles.tile([128, 2, 96], f16, tag="e3", name="e3")
        esc_sb = e3[:, :, 0:32]
        w16 = e3[:, :, 32:64]
        ctxT_sb = e3[:, :, 64:96]
        zpack = singles.tile([1, 2, 48], f32, tag="zpack", name="zpack")
        zrow = zpack[:, :, 0:32]
        zb = zpack[:, :, 32:40]
        rz = zpack[:, :, 40:48]
        rz16 = singles.tile([1, 2, 8], f16, tag="rz16", name="rz16")
        junk = singles.tile([128, DEC], f16, tag="junk", name="junk")
        ctx_line = singles.tile([1, 2, ENC], f16, tag="ctx_line", name="ctx_line")

        def stt(out, in0, scalar, in1, op0, op1, accum_out=None):
            nc.vector.scalar_tensor_tensor(
                out=out, in0=in0, scalar=scalar, in1=in1, op0=op0, op1=op1,
                accum_out=accum_out)

        def chunk_act(ps, ch):
            scale = 0.5 if ch < 6 else 1.0
            nc.scalar.activation(
                t_sb[:, ch * 512:(ch + 1) * 512], ps[:], AF.Tanh, scale=scale)

        def lstm_tail(C_state, HT_state, wslot):
            ti, tf = t_sb[:, 0:DEC], t_sb[:, DEC:2 * DEC]
            to, tg = t_sb[:, 2 * DEC:3 * DEC], t_sb[:, 3 * DEC:4 * DEC]
            stt(As_[:], tf, 1.0, C_state[:], ALU.add, ALU.mult)
            stt(Bs_[:], ti, 1.0, tg, ALU.add, ALU.mult)
            stt(C_state[:], As_[:], 0.5, Bs_[:], ALU.mult, ALU.add)
            nc.scalar.activation(tanhc, C_state[:], AF.Tanh, scale=0.5)
            stt(Hnew, to, 1.0, tanhc, ALU.add, ALU.mult)
            tp = tp_pool.tile([128, 64], f16, tag="tp", name="tp")
            for j in range(8):
                nc.tensor.transpose(
                    tp[:, 8 * j:8 * j + 8], Hnew[:, 128 * j:128 * (j + 1)],
                    ident_sb[0:BL, 0:BL])
            nc.vector.tensor_copy(HT_state[:, wslot, :], tp[:])

        with (
            tc.tile_pool(name="stream", bufs=4) as streamp,
            tc.tile_pool(name="pkenc", bufs=2) as pkp,
            tc.tile_pool(name="es", bufs=2) as esp,
            tc.tile_pool(name="tp", bufs=2, space="PSUM") as tp_pool,
            tc.tile_pool(name="bank", bufs=4, space="PSUM") as bankp,
            tc.tile_pool(name="rep", bufs=1, space="PSUM") as repp,
        ):
            with tc.For_i(0, T, 2) as tv:
                for u in range(2):
                    rslot, wslot = 1 - u, u
                    # ======== attention LSTM ========
                    xga_t = pkp.tile([BL, G], f16, tag="pkenc", name="xga_t")
                    nc.sync.dma_start(xga_t[:], dr_xga[ds((tv + u) * BL, BL)])
                    for grp in range(2):
                        chunks = [bankp.tile([BL, 512], f32, tag="bank",
                                             name=f"gch{ci}") for ci in range(4)]
                        for kt in range(8):
                            w = streamp.tile([128, 2048], f16, tag="stream",
                                             name="wha")
                            nc.sync.dma_start(
                                w[:], dr_whha[kt, :, 2048 * grp:2048 * (grp + 1)])
                            for ci in range(4):
                                nc.tensor.matmul(
                                    chunks[ci][:],
                                    ahT[:, rslot, 8 * kt:8 * kt + 8],
                                    w[:, ci * 512:(ci + 1) * 512],
                                    start=(kt == 0), stop=False)
                        for ci in range(4):
                            ch = grp * 4 + ci
                            nc.tensor.matmul(
                                chunks[ci][:], ident_sb[0:BL, 0:BL],
                                xga_t[:, ch * 512:(ch + 1) * 512],
                                start=False, stop=True)
                            chunk_act(chunks[ci], ch)
                    lstm_tail(Ca, ahT, wslot)
                    # ======== pq ========
                    pqc = [bankp.tile([BL, 512], f32, tag="bank",
                                      name=f"pqc{ci}") for ci in range(2)]
                    for kt in range(8):
                        wq_t = streamp.tile([128, DEC], f16, tag="stream",
                                            name="wq_t")
                        nc.sync.dma_start(wq_t[:], dr_wq[kt])
                        for ci in range(2):
                            nc.tensor.matmul(
                                pqc[ci][:], ahT[:, wslot, 8 * kt:8 * kt + 8],
                                wq_t[:, ci * 512:(ci + 1) * 512],
                                start=(kt == 0), stop=(kt == 7))
                    for ci in range(2):
                        nc.vector.tensor_copy(
                            pq_sb[:, u, ci * 512:(ci + 1) * 512], pqc[ci][:])
                    nc.sync.dma_start(pq_row[32 * u:32 * u + 1, :],
                                      pq_sb[:, u, :])

                # ======== energy + scores (pk shared by u=0,1) ========
                for I in range(BL):
                    pk_t = pkp.tile([128, 4 * DEC], f16, tag="pkenc",
                                    name="pk_t")
                    nc.sync.dma_start(pk_t[:], dr_pk2[I])
                    for u in range(2):
                        rep = repp.tile([128, DEC], f32, tag="rep", name="rep")
                        for rh in range(2):
                            nc.tensor.matmul(
                                rep[:, rh * 512:(rh + 1) * 512],
                                ones_r_sb[32 * u:32 * u + 1, :],
                                pq_row[32 * u:32 * u + 1,
                                       DEC * I + rh * 512:DEC * I + (rh + 1) * 512],
                                start=True, stop=True)
                        ra = rep[:]
                        rep_b = bass.AP(ra.tensor, ra.offset,
                                        [ra.ap[0], [0, 2], [1, DEC]])
                        for hf in range(2):
                            pk_v = pk_t[:, (2 * hf) * DEC:(2 * hf + 2) * DEC]
                            pk_v = pk_v.rearrange("p (s d) -> p s d", s=2)
                            es = esp.tile([128, 2 * DEC], f16, tag="es",
                                          name="es")
                            es_v = es[:].rearrange("p (s d) -> p s d", s=2)
                            nc.vector.tensor_add(es_v, pk_v, rep_b)
                            nc.scalar.activation(es[:], es[:], AF.Tanh)
                            for sb in range(2):
                                col = 4 * I + 2 * hf + sb
                                stt(junk[:], es_v[:, sb, :], 1.0, werep_sb,
                                    ALU.mult, ALU.mult,
                                    accum_out=sc_sb[:, u, col:col + 1])

                # ======== softmax ========
                for u in range(2):
                    nc.scalar.activation(esc_sb[:, u, :], sc_sb[:, u, :], AF.Exp)
                    zp = tp_pool.tile([1, 32], f32, tag="tp", name="zp")
                    nc.tensor.matmul(zp[:], ones_c_sb, esc_sb[:, u, :],
                                     start=True, stop=True)
                    nc.vector.tensor_copy(zrow[:, u, :], zp[:])
                    nc.vector.tensor_reduce(
                        out=zb[:, u, :],
                        in_=zrow[:, u, :].rearrange("p (b s) -> p b s", b=8),
                        axis=AX.X, op=ALU.add)
                    nc.vector.reciprocal(rz[:, u, :], zb[:, u, :])
                    nc.vector.tensor_copy(rz16[:, u, :], rz[:, u, :])
                    ra = rz16[:, u, :]
                    rz_b = bass.AP(ra.tensor, ra.offset,
                                   [ra.ap[0], [1, 8], [0, 4]])
                    rzp = repp.tile([128, 32], f32, tag="rep", name="rzp")
                    nc.tensor.matmul(rzp[:], ones_r_sb[0:1, :], rz_b,
                                     start=True, stop=True)
                    nc.vector.tensor_mul(w16[:, u, :], esc_sb[:, u, :], rzp[:])

                # ======== context (enc shared by u) ========
                tpcs = [tp_pool.tile([128, 64], f16, tag="tp", name=f"tpc{u}")
                        for u in range(2)]
                for b in range(BL):
                    enc_t = pkp.tile([128, 4 * ENC], f16, tag="pkenc",
                                     name="enc_t")
                    nc.sync.dma_start(enc_t[:], dr_enc[b])
                    enc_v = enc_t[:].rearrange("p (s e) -> p s e", s=4)
                    for u in range(2):
                        cps = bankp.tile([1, ENC], f32, tag="bank", name="cps")
                        for st in range(4):
                            nc.tensor.matmul(
                                cps[:], w16[:, u, 4 * b + st:4 * b + st + 1],
                                enc_v[:, st, :],
                                start=(st == 0), stop=(st == 3))
                        nc.vector.tensor_copy(ctx_line[:, u, :], cps[:])
                        for st in range(4):
                            k = 8 * st + b
                            nc.tensor.transpose(
                                tpcs[u][:, 2 * k:2 * k + 1],
                                ctx_line[:, u, st * 128:(st + 1) * 128],
                                ident_sb[0:1, 0:1])
                for u in range(2):
                    ta = tpcs[u][:]
                    ts2 = bass.AP(ta.tensor, ta.offset, [ta.ap[0], [2, 32]])
                    nc.vector.tensor_copy(ctxT_sb[:, u, :], ts2)

                # ======== decoder LSTM (u sequential) ========
                for u in range(2):
                    rslot, wslot = 1 - u, u
                    for grp in range(2):
                        glo = grp * 2048
                        chunks = [bankp.tile([BL, 512], f32, tag="bank",
                                             name=f"dch{ci}") for ci in range(4)]
                        for kt in range(8):  # ah-part
                            if kt < RES_A:
                                rhs_full = wihda_sb[:, kt * G + glo:
                                                    kt * G + glo + 2048]
                            else:
                                w = streamp.tile([128, 2048], f16, tag="stream",
                                                 name="wda")
                                nc.sync.dma_start(
                                    w[:], dr_wihda[kt, :, glo:glo + 2048])
                                rhs_full = w[:]
                            for ci in range(4):
                                nc.tensor.matmul(
                                    chunks[ci][:],
                                    ahT[:, u, 8 * kt:8 * kt + 8],
                                    rhs_full[:, ci * 512:(ci + 1) * 512],
                                    start=(kt == 0), stop=False)
                        for kt in range(4):  # context-part
                            w = streamp.tile([128, 2048], f16, tag="stream",
                                             name="wdc")
                            nc.sync.dma_start(
                                w[:], dr_wihdc[kt, :, glo:glo + 2048])
                            for ci in range(4):
                                nc.tensor.matmul(
                                    chunks[ci][:],
                                    ctxT_sb[:, u, 8 * kt:8 * kt + 8],
                                    w[:, ci * 512:(ci + 1) * 512],
                                    start=False, stop=False)
                        for kt in range(8):  # recurrent part
                            for ci in range(4):
                                last = (kt == 7) and not has_bias_d
                                nc.tensor.matmul(
                                    chunks[ci][:],
                                    dhT[:, rslot, 8 * kt:8 * kt + 8],
                                    whhd_sb[:, kt * G + glo + ci * 512:
                                            kt * G + glo + (ci + 1) * 512],
                                    start=False, stop=last)
                        for ci in range(4):
                            ch = grp * 4 + ci
                            if has_bias_d:
                                nc.tensor.matmul(
                                    chunks[ci][:], ones_r_sb[0:1, 0:BL],
                                    biasd_sb[:, ch * 512:(ch + 1) * 512],
                                    start=False, stop=True)
                            chunk_act(chunks[ci], ch)
                    lstm_tail(Cd, dhT, wslot)
                    nc.sync.dma_start(dr_dhT[ds((tv + u) * 128, 128)],
                                      dhT[:, wslot, :])

        # ---- deferred mel projection ----
        with (
            tc.tile_pool(name="melr", bufs=4) as melr,
            tc.tile_pool(name="melps", bufs=2, space="PSUM") as melps,
            tc.tile_pool(name="melsb", bufs=2) as melsb,
        ):
            NTOT = T * BL
            dhT_v = dr_dhT[:].rearrange("(t p) b -> t p b", p=128)
            col = 0
            while col < NTOT:
                n = min(512, NTOT - col)
                t0, nt = col // BL, n // BL
                ps = melps.tile([MEL, 512], f32, tag="mps", name="mps")
                for kt in range(8):
                    r = melr.tile([128, 512], f16, tag="mel", name="mr")
                    src = dhT_v[t0:t0 + nt, :, 8 * kt:8 * kt + 8]
                    nc.sync.dma_start(
                        r[:, 0:n].rearrange("p (t b) -> p t b", t=nt),
                        src.rearrange("t p b -> p t b"))
                    nc.tensor.matmul(
                        ps[:, 0:n], wmelT_sb[:, MEL * kt:MEL * (kt + 1)],
                        r[:, 0:n], start=(kt == 0), stop=(kt == 7))
                o = melsb.tile([MEL, 512], f32, tag="mo", name="mo")
                nc.scalar.activation(o[:, 0:n], ps[:, 0:n], AF.Identity,
                                     bias=bmel_sb[:, 0:1])
                nc.sync.dma_start(dr_mel[:, col:col + n], o[:, 0:n])
                col += n

    nc.compile()
    return nc


_prog_cache = {}


def _get_prog(T, has_bias_d):
    key = (T, has_bias_d, RES_A)
    if key not in _prog_cache:
        _prog_cache[key] = build_program(T, has_bias_d)
    return _prog_cache[key]


def run(inputs, T=T_FULL, trace=False):
    from concourse.bass_utils import run_bass_kernel_spmd

    meta, per_core = prep_host(inputs, T)
    nc = _get_prog(T, meta["has_bias_d"])
    res = run_bass_kernel_spmd(nc, per_core, list(range(NCORES)), trace=trace)
    outs = []
    for c in range(NCORES):
        m = np.asarray(res.results[c]["melT"], F32)
        outs.append(m.reshape(MEL, T, BL).transpose(2, 1, 0))
    return np.concatenate(outs, axis=0), res


def kernel(**inputs):
    out, _ = run(inputs, T=T_FULL)
    return np.ascontiguousarray(out.astype(np.float32))

